# revision 31
# baseline (speedup 1.0000x reference)
"""MoE routing kernel (nn_DecFCSwitch) for 8 Trainium2 NeuronCores.

Reference computes all 16 expert branches for every token and then
selects one per token.  Only the selected branch matters, so:

  host:   sort tokens by expert, pad each expert's tokens to capacity C,
          relu(x) (the residual add also stays on host), transpose so
          the feature dim lands on SBUF partitions, cast to bf16.
  device: expert-parallel SPMD — core i owns experts {2i, 2i+1} and runs
          a 2-layer MLP (no inter-layer activation) on its experts'
          tokens.  All matmuls keep tokens on the PSUM free dim, so the
          per-expert biases are plain per-partition broadcasts.
  host:   transpose back, scatter rows to token order, out = x + sel.

Compute dtype bf16 (PSUM accumulates fp32); biases/output fp32 paths.
"""

import os
import sys

import numpy as np

for _p in ("/opt/trn_rl_repo", "/root/.axon_site/_ro/trn_rl_repo"):
    if os.path.isdir(_p) and _p not in sys.path:
        sys.path.insert(0, _p)

import ml_dtypes

B, D, S, NB = 4096, 1024, 256, 16
NCORES = 8
EPC = NB // NCORES  # experts per core
KD = D // 128  # d-dim k/m tiles
KS = S // 128  # s-dim tiles

BF16 = ml_dtypes.bfloat16

_programs = {}  # C -> compiled Bacc program
LAST_RESULT = None  # BassKernelResults of the most recent run (for test.py)


N_WARM = 36  # PE warm-up matmuls (HAM p-state ramp) before real data lands


def _build_program(C):
    import concourse.mybir as mybir
    import concourse.tile as tile
    from concourse import bacc

    cdt = mybir.dt.bfloat16
    f32 = mybir.dt.float32
    ident = mybir.ActivationFunctionType.Identity

    nc = bacc.Bacc()
    hT = nc.declare_dram_parameter("hT", [KD, 128, EPC * C], cdt, isOutput=False)
    w1 = nc.declare_dram_parameter("w1", [EPC, KD, 128, S], cdt, isOutput=False)
    w2 = nc.declare_dram_parameter("w2", [EPC, KS, 128, D], cdt, isOutput=False)
    # b_in and b_out packed: bc[e, 0:KS] = b_in tiles, bc[e, KS:KS+KD] = b_out
    bc = nc.declare_dram_parameter("bc", [EPC, KS + KD, 128, 1], f32, isOutput=False)
    yT = nc.declare_dram_parameter("yT", [KD, 128, EPC * C], cdt, isOutput=True)

    HK = KD // 2  # h is loaded in two half-loads of HK d-tiles each

    with tile.TileContext(nc) as tc:
        with (
            tc.tile_pool(name="bias", bufs=1) as bias_pool,
            tc.tile_pool(name="h", bufs=1) as h_pool,
            tc.tile_pool(name="w1p", bufs=1) as w1_pool,
            tc.tile_pool(name="w2p", bufs=1) as w2_pool,
            tc.tile_pool(name="hid", bufs=4) as hid_pool,
            tc.tile_pool(name="yout", bufs=2) as y_pool,
            tc.tile_pool(name="ps1", bufs=2, space="PSUM") as ps1_pool,
            tc.tile_pool(name="ps2", bufs=5, space="PSUM") as ps2_pool,
            tc.tile_pool(name="warm", bufs=1) as warm_pool,
            tc.tile_pool(name="warmps", bufs=1, space="PSUM") as warm_ps_pool,
        ):
            # Dummy matmuls keep the PE busy from t=0 so the HAM throttle is
            # fully ramped by the time the first real operands arrive.
            wz = warm_pool.tile([128, 64], cdt, tag="wz")
            nc.gpsimd.memset(wz[:], 0)
            wps = warm_ps_pool.tile([128, 64], f32, tag="wps")
            for _ in range(N_WARM):
                nc.tensor.matmul(
                    wps[0:64, :], lhsT=wz[:, 0:64], rhs=wz[:], start=True, stop=True
                )
            # Biases ride the SWDGE (gpsimd) path: the HWDGE rings are the
            # serial resource, Pool is idle.
            NB_COL = KS + KD
            bct = bias_pool.tile([128, EPC * NB_COL], f32, tag="bc")
            nc.gpsimd.dma_start(
                out=bct[:].rearrange("p (e t) -> p e t", e=EPC),
                in_=bc[:, :, :, 0].rearrange("e t p -> p e t"),
            )

            def b1_ap(e, t):
                return bct[:, e * NB_COL + t : e * NB_COL + t + 1]

            def b2_ap(e, k):
                return bct[:, e * NB_COL + KS + k : e * NB_COL + KS + k + 1]

            # Activations: four quarter-loads (2 d-tiles each) so the PE can
            # start as soon as the first chunks land.  Shared by both experts.
            HQ = KD // 4  # d-tiles per h chunk
            h_pool_tiles = [
                h_pool.tile([128, HQ * EPC * C], cdt, tag=f"h{q}", name=f"h{q}")
                for q in range(4)
            ]

            def load_h(q):
                nc.sync.dma_start(
                    out=h_pool_tiles[q][:].rearrange("p (k n) -> p k n", k=HQ),
                    in_=hT[q * HQ : (q + 1) * HQ].rearrange("k p n -> p k n"),
                )

            def h_slice(k, e):  # rhs [128, C] for d-tile k, expert e
                base = (k % HQ) * EPC * C + e * C
                return h_pool_tiles[k // HQ][:, base : base + C]

            def make_w1(e):
                return w1_pool.tile([128, KD * S], cdt, tag=f"w1_{e}", name=f"w1_{e}")

            def load_w1_half(e, w1t, kh):
                nc.sync.dma_start(
                    out=w1t[:, kh * HK * S : (kh + 1) * HK * S].rearrange(
                        "p (k s) -> p k s", k=HK
                    ),
                    in_=w1[e][kh * HK : (kh + 1) * HK].rearrange("k p s -> p k s"),
                )

            # w2 tile free layout: (mh, t, d_within_half), so a d-half can be
            # loaded on its own and layer 2's first m-groups unblock sooner.
            DH = D // 2

            def load_w2(e, w2t):
                nc.sync.dma_start(
                    out=w2t[:].rearrange("p (mh t d) -> p mh t d", mh=2, t=KS),
                    in_=w2[e].rearrange("t p (mh d) -> p mh t d", mh=2),
                )

            def load_w2_half(e, w2t, mh):
                nc.sync.dma_start(
                    out=w2t[:, mh * KS * DH : (mh + 1) * KS * DH].rearrange(
                        "p (t d) -> p t d", t=KS
                    ),
                    in_=w2[e][:, :, mh * DH : (mh + 1) * DH].rearrange("t p d -> p t d"),
                )

            def w2_slice(w2t, t, m):
                mh, dd = divmod(m * 128, DH)
                return w2t[:, mh * KS * DH + t * DH + dd : mh * KS * DH + t * DH + dd + 128]

            # SP ring, in first-need order, fine-grained at the start so the
            # first matmuls unblock as early as possible.
            w1_tiles = [make_w1(0), make_w1(1)]
            w2_tiles = [
                w2_pool.tile([128, KS * D], cdt, tag=f"w2_{e}", name=f"w2_{e}")
                for e in range(EPC)
            ]
            load_h(0)
            load_w1_half(0, w1_tiles[0], 0)
            load_h(1)
            load_h(2)
            load_w1_half(0, w1_tiles[0], 1)
            load_h(3)

            for e in range(EPC):
                # Layer 1: hid^T[s, c] = sum_d W_in[s, d] * h^T[d, c]
                hids = []
                for t in range(KS):
                    ps = ps1_pool.tile([128, C], f32)
                    for k in range(KD):
                        nc.tensor.matmul(
                            ps[:],
                            lhsT=w1_tiles[e][:, k * S + t * 128 : k * S + t * 128 + 128],
                            rhs=h_slice(k, e),
                            start=(k == 0),
                            stop=(k == KD - 1),
                        )
                    if e == 0 and t == 0:
                        load_w2(0, w2_tiles[0])
                    elif e == 0 and t == 1:
                        load_w1_half(1, w1_tiles[1], 0)
                        load_w1_half(1, w1_tiles[1], 1)
                    hid = hid_pool.tile([128, C], cdt)
                    nc.scalar.activation(hid[:], ps[:], ident, bias=b1_ap(e, t))
                    hids.append(hid)

                # Layer 2: y^T[d, c] = sum_s W_out[d, s] * hid^T[s, c]
                # Evictions alternate ACT / DVE into one [128, KD*C] tile;
                # stored in two strided half-DMAs so the tail store is short.
                y_big = y_pool.tile([128, KD * C], cdt)
                for m in range(KD):
                    ps = ps2_pool.tile([128, C], f32)
                    for t in range(KS):
                        nc.tensor.matmul(
                            ps[:],
                            lhsT=w2_slice(w2_tiles[e], t, m),
                            rhs=hids[t][:],
                            start=(t == 0),
                            stop=(t == KS - 1),
                        )
                    if e == 0 and m == 0:
                        load_w2_half(1, w2_tiles[1], 0)
                    elif e == 0 and m == 2:
                        load_w2_half(1, w2_tiles[1], 1)
                    dst = y_big[:, m * C : (m + 1) * C]
                    bias_ap = b2_ap(e, m)
                    if m % 2 == 0:
                        nc.scalar.activation(dst, ps[:], ident, bias=bias_ap)
                    else:
                        nc.vector.tensor_scalar_add(dst, ps[:], bias_ap)
                    # Stores alternate between the two idle DMA issuers —
                    # gpsimd (SWDGE) and SP (HWDGE, free once loads are done) —
                    # so tail stores don't serialize on one generator.
                    store_after = {3: (0, 4), 7: (4, 8)} if e == 0 else {
                        1: (0, 2), 3: (2, 4), 5: (4, 6), 7: (6, 8)
                    }
                    if m in store_after:
                        k0, k1 = store_after[m]
                        issuer = nc.gpsimd if (m // 2) % 2 == 0 else nc.sync
                        issuer.dma_start(
                            out=yT[k0:k1, :, e * C : (e + 1) * C]
                            .rearrange("k p n -> p k n"),
                            in_=y_big[:, k0 * C : k1 * C]
                            .rearrange("p (k n) -> p k n", k=k1 - k0),
                        )

    nc.compile()
    return nc


def kernel(x, y_index, W_in, b_in, W_out, b_out):
    global LAST_RESULT
    from concourse.bass_utils import run_bass_kernel_spmd

    x = np.asarray(x, dtype=np.float32)
    W_in = np.asarray(W_in, dtype=np.float32)
    b_in = np.asarray(b_in, dtype=np.float32)
    W_out = np.asarray(W_out, dtype=np.float32)
    b_out = np.asarray(b_out, dtype=np.float32)
    eidx = np.asarray(y_index).reshape(-1).astype(np.int64)

    counts = np.bincount(eidx, minlength=NB)
    C = max(280, int(-(-counts.max() // 8) * 8))  # capacity per expert

    # --- host dispatch: group tokens by expert ---------------------------
    order = np.argsort(eidx, kind="stable")
    starts = np.zeros(NB + 1, dtype=np.int64)
    np.cumsum(counts, out=starts[1:])

    h = np.maximum(x, 0.0)
    Xg = np.zeros((NB, C, D), dtype=np.float32)
    for e in range(NB):
        toks = order[starts[e] : starts[e + 1]]
        Xg[e, : counts[e]] = h[toks]

    # [NB, C, D] -> per core [D, EPC*C] -> [KD, 128, EPC*C]
    hT_all = (
        Xg.reshape(NCORES, EPC * C, D)
        .transpose(0, 2, 1)
        .reshape(NCORES, KD, 128, EPC * C)
        .astype(BF16)
    )
    w1_all = (
        W_in.transpose(0, 2, 1).reshape(NCORES, EPC, KD, 128, S).astype(BF16)
    )
    w2_all = (
        W_out.transpose(0, 2, 1).reshape(NCORES, EPC, KS, 128, D).astype(BF16)
    )
    bc_all = np.concatenate(
        [b_in.reshape(NB, KS, 128, 1), b_out.reshape(NB, KD, 128, 1)], axis=1
    ).reshape(NCORES, EPC, KS + KD, 128, 1)

    if C not in _programs:
        _programs[C] = _build_program(C)
    nc = _programs[C]

    in_maps = [
        {
            "hT": np.ascontiguousarray(hT_all[i]),
            "w1": np.ascontiguousarray(w1_all[i]),
            "w2": np.ascontiguousarray(w2_all[i]),
            "bc": np.ascontiguousarray(bc_all[i]),
        }
        for i in range(NCORES)
    ]

    trace = bool(int(os.environ.get("KERNEL_TRACE", "0")))
    res = run_bass_kernel_spmd(nc, in_maps, list(range(NCORES)), trace=trace)
    LAST_RESULT = res

    # --- host gather: transpose back, scatter to token order -------------
    out = np.empty_like(x)
    Yg = np.stack(
        [
            r["yT"].reshape(D, EPC * C).astype(np.float32)
            for r in res.results
        ]
    )  # [NCORES, D, EPC*C]
    Yg = Yg.transpose(0, 2, 1).reshape(NB, C, D)
    for e in range(NB):
        toks = order[starts[e] : starts[e + 1]]
        out[toks] = x[toks] + Yg[e, : counts[e]]
    return out
